# revision 5
# baseline (speedup 1.0000x reference)
"""GAT (graph attention) layer on 8 Trainium2 NeuronCores.

Reference computation (N=8192, F_IN=256, F_OUT=64, alpha=0.2):
    Wh     = h @ W                                  [N, 64]
    f_src  = Wh @ a[:64, 0]                         [N]
    f_dst  = Wh @ a[64:, 0]                         [N]
    e      = leaky_relu(f_src[:,None] + f_dst[None,:], 0.2)
    att    = softmax(where(adj > 0, e, -9e15), axis=1)
    out    = elu(att @ Wh)

Sharding: row-shard the N query dimension across 8 cores (1024 rows each).
Each core gets its adjacency shard transposed and cast to bf16 on the host
(adjT[j,i] = adj[i,j]; exact for 0/1) plus the full hT and W/WT in bf16/f32.

Device algebra ("clamp-at-1"):
  exp(lrelu(x)) with x = fs_i + fd_j factors exactly as
     exp(0.2 fs_i) * exp(0.2 fd_j) * max(u_i * g_j, 1),
     u_i = exp(0.8 fs_i), g_j = exp(0.8 fd_j).
  The row factor exp(0.2 fs_i) cancels in the softmax; gp_j = exp(0.2 fd_j)
  is folded into the matmul lhs: rhs_aug[j,:] = gp_j * [Wh_j | 1], whose 65th
  column simultaneously produces the softmax denominator Z.

  Per j-chunk ([j=128 partitions, i=1024 free], all bf16):
    Xm = max(u_b * g_j, 1.0)  one DVE tensor_scalar with ONE per-partition AP
                              scalar + an immediate -> 4x perf mode (~490ns);
                              (the two-AP-scalar variant would run 1x ~1600ns)
    p  = Xm * adjT            tensor_tensor, quad-batched [128,4,1024] (2x
                              mode, ~570ns/chunk effective); all elementwise
                              work stays on DVE: offloading to Pool costs more
                              in SBUF-port contention (measured ~0.8us DVE
                              inflation per us of concurrent Pool work) than
                              Pool contributes.
    accT[f,i] += rhs_aug[j,f]^T @ p[j,i]   2 bf16 matmuls per chunk

  Structure: a software-pipelined group loop (4 chunks/group) with
  production (Wh matmuls + ACT exp/scale-copies into rhs_aug/gmat) running
  LOOK=2 groups ahead of attention (combiners + quad mask + acc matmuls),
  which keeps the PE fed (its clock ramps 1.2->2.4 GHz only after 3us of
  continuous execution) and overlaps the adjacency stream (16 MiB/core,
  ~60% of the memory-roofline traffic) with compute.  The first/last groups
  use per-chunk masks to shorten pipeline ramp and drain.

  Prologue: w_src/w_dst come from one matmul against host-provided WT
  (no PE transposes); own-row f_src is computed as a single-partition row
  [1,1024] via wsrc^T @ hsT, exp'd on ACT, and replicated to all partitions
  with gpsimd partition_broadcast (no DRAM bounce).  Epilogue: one
  PSUM->SBUF copy per accumulator half, 4 PE transposes each, batched
  [128,4,64] vector ops; elu(x) = max(exp(min(x,0)) - 1, x).
"""

import sys

sys.path.insert(0, "/opt/trn_rl_repo")

import os

import numpy as np
import ml_dtypes

import concourse.bass as bass  # noqa: F401
import concourse.mybir as mybir
import concourse.tile as tile
from concourse import bacc
from concourse.bass_utils import run_bass_kernel_spmd
from concourse.masks import make_identity

N = 8192
F_IN = 256
F_OUT = 64
N_CORES = 8
ROWS = N // N_CORES  # 1024 query rows per core
MCH = N // 128  # 64 j-chunks
LCH = ROWS // 128  # 8 i-blocks
KCH = F_IN // 128  # 2 contraction chunks

F32 = mybir.dt.float32
BF16 = mybir.dt.bfloat16
I32 = mybir.dt.int32
Act = mybir.ActivationFunctionType
Alu = mybir.AluOpType
NPBF16 = ml_dtypes.bfloat16

_CACHE = {}


def _build_nc():
    nc = bacc.Bacc(
        "TRN2",
        target_bir_lowering=False,
        debug=False,
        enable_asserts=False,
        num_devices=N_CORES,
    )

    hT = nc.dram_tensor("hT", [F_IN, N], BF16, kind="ExternalInput")
    hsT = nc.dram_tensor("hsT", [F_IN, ROWS], BF16, kind="ExternalInput")
    adjT = nc.dram_tensor("adjT", [N, ROWS], BF16, kind="ExternalInput")
    W = nc.dram_tensor("W", [F_IN, F_OUT], F32, kind="ExternalInput")
    WT = nc.dram_tensor("WT", [F_OUT, F_IN], F32, kind="ExternalInput")
    a = nc.dram_tensor("a", [2 * F_OUT, 1], F32, kind="ExternalInput")
    out = nc.dram_tensor("out", [ROWS, F_OUT], F32, kind="ExternalOutput")
    fsd = nc.dram_tensor("fsd", [ROWS], BF16)  # u = exp(.8 fs) bounce

    with tile.TileContext(nc) as tc:
        _kernel_body(nc, tc, hT, hsT, adjT, W, WT, a, out, fsd)

    nc.compile()
    return nc


def _kernel_body(nc, tc, hT, hsT, adjT, W, WT, a, out, fsd):
    ADJBUFS = int(os.environ.get("GAT_ADJBUFS", "6"))  # 4-chunk tiles
    WKBUFS = int(os.environ.get("GAT_WKBUFS", "3"))
    PBC = int(os.environ.get("GAT_PBC", "1"))  # u_b via partition_broadcast
    GRP = 4  # chunks per group
    EPBUFS = 4
    # quad-mask engine split: which group slots (mod 8) go to Pool
    slots_env = os.environ.get("GAT_SLOTS", "")
    pool_slots = {int(x) for x in slots_env.split(",")} if slots_env else set()
    POOL16 = int(os.environ.get("GAT_POOL16", "-1"))  # extra pool slot per 16
    SUBSZ = int(os.environ.get("GAT_SUBSZ", "4"))  # chunks per adjT DMA
    RHSDVE = int(os.environ.get("GAT_RHSDVE", "0"))  # rhs scale-copy on DVE

    with (
        tc.tile_pool(name="consts", bufs=1) as consts,
        tc.tile_pool(name="adjp", bufs=ADJBUFS) as adjp,
        tc.tile_pool(name="wk", bufs=WKBUFS) as wk,
        tc.tile_pool(name="ep", bufs=EPBUFS) as ep,
        tc.tile_pool(name="psS", bufs=2, space="PSUM") as psS,
        tc.tile_pool(name="psF", bufs=1, space="PSUM") as psF,
        tc.tile_pool(name="psW", bufs=2, space="PSUM") as psW,
        tc.tile_pool(name="psACC", bufs=1, space="PSUM") as psACC,
    ):
        # ---------------- prologue: own-row u broadcast first ----------
        # hsT DMA leads the sync queue; WT/a/W head the scalar queue, so the
        # u_b chain (wsrc -> fso row -> exp -> broadcast) completes before
        # the adjacency flood saturates the DMA engines.
        hsTs = consts.tile([128, 2, ROWS], BF16)
        for kc in range(KCH):
            nc.sync.dma_start(
                out=hsTs[:, kc, :], in_=hsT[kc * 128 : (kc + 1) * 128, :]
            )

        # Waug = [W | w_src | w_dst]: w_{src,dst}[k] = sum_f WT[f,k] a[f,:]
        WTt = consts.tile([64, 2, 128], F32)
        nc.scalar.dma_start(
            out=WTt, in_=WT[:, :].rearrange("f (c p) -> f c p", p=128)
        )
        a2 = consts.tile([64, 2], F32)
        nc.scalar.dma_start(out=a2, in_=a.rearrange("(c p) x -> p (c x)", p=F_OUT))

        # warm the gpsimd ucode lib used by partition_broadcast
        pbw_src = consts.tile([1, 8], BF16)
        nc.gpsimd.memset(pbw_src, 0.0)
        pbw_dst = consts.tile([128, 8], BF16)
        nc.gpsimd.partition_broadcast(pbw_dst, pbw_src)

        wab = consts.tile([128, 2, 2], BF16)  # [p, kc, (src, dst)] bf16
        wps_list = []
        for kc in range(KCH):
            scrw = psS.tile([128, 4, 65], F32, tag="s")
            wps = scrw[:, 0, 0:2]
            nc.tensor.matmul(wps, lhsT=WTt[:, kc, :], rhs=a2, start=True, stop=True)
            nc.vector.tensor_copy(wab[:, kc, :], wps)
            wps_list.append(wps)

        # own-row f_src as a single-partition row: fso[0, i] = hs_i . wsrc
        # exp of each half issues as soon as that half's matmuls finish
        fso_ps = psF.tile([1, ROWS], F32, tag="fso")
        uo_row = consts.tile([1, ROWS], BF16)
        for half in range(2):
            hs = slice(half * 512, (half + 1) * 512)
            for kc in range(KCH):
                nc.tensor.matmul(
                    fso_ps[:, hs],
                    lhsT=wab[:, kc, 0:1],
                    rhs=hsTs[:, kc, hs],
                    start=(kc == 0),
                    stop=(kc == KCH - 1),
                )
            nc.scalar.activation(
                uo_row[:, hs], fso_ps[:, hs], Act.Exp, bias=0.0, scale=0.8
            )
        u_b = consts.tile([128, ROWS], BF16)
        if PBC:
            nc.gpsimd.partition_broadcast(u_b, uo_row)
        else:
            nc.gpsimd.dma_start(out=fsd[:], in_=uo_row)
            fsd_bc = bass.AP(tensor=fsd, offset=0, ap=[[0, 128], [1, ROWS]])
            nc.vector.dma_start(out=u_b, in_=fsd_bc)

        # W body + Waug assembly after the u_b chain so queue-counter waits
        # on the scalar DMA queue don't gate the wsrc matmuls.
        Waug = consts.tile([128, 2, 66], F32)
        nc.scalar.dma_start(
            out=Waug[:, :, 0:F_OUT],
            in_=W[:, :].rearrange("(c p) f -> p c f", p=128),
        )
        for kc in range(KCH):
            nc.vector.tensor_copy(Waug[:, kc, F_OUT : F_OUT + 2], wps_list[kc])
        Waug_bf = consts.tile([128, 2, 66], BF16)
        nc.vector.tensor_copy(Waug_bf, Waug)

        idf = consts.tile([128, 128], F32)
        make_identity(nc, idf)

        # ---------------- main loop ------------------------------------
        hTs = consts.tile([128, 2, N], BF16)
        gmat = consts.tile([128, MCH], F32)  # exp(0.8 fd_j)
        gpmat = consts.tile([128, MCH], F32)  # exp(0.2 fd_j)
        rhs_aug = consts.tile([128, MCH, 65], BF16)

        accL = psACC.tile([65, 512], F32, tag="accL")
        accR = psACC.tile([65, 512], F32, tag="accR")

        NG = MCH // GRP  # 16 groups
        LOOK = int(os.environ.get("GAT_LOOK", "2"))  # production lookahead
        adjt_tiles = {}

        for kc in range(KCH):
            nc.scalar.dma_start(
                out=hTs[:, kc, 0:2048],
                in_=hT[kc * 128 : (kc + 1) * 128, 0:2048],
            )

        def produce(gi):
            mc0 = gi * GRP
            # prefetch next hTs slice two groups before it is needed
            if gi % 4 == 2 and gi // 4 < 3:
                s1 = gi // 4 + 1
                for kc in range(KCH):
                    nc.scalar.dma_start(
                        out=hTs[:, kc, s1 * 2048 : (s1 + 1) * 2048],
                        in_=hT[kc * 128 : (kc + 1) * 128, s1 * 2048 : (s1 + 1) * 2048],
                    )
            adjt = adjp.tile([128, GRP, ROWS], BF16, tag="adj")
            adjt_tiles[gi] = adjt
            sub = 1 if mc0 == MCH - GRP else SUBSZ
            for h2 in range(GRP // sub):
                m2 = mc0 + h2 * sub
                nc.sync.dma_start(
                    out=adjt[:, h2 * sub : (h2 + 1) * sub, :],
                    in_=adjT[m2 * 128 : (m2 + sub) * 128, :].rearrange(
                        "(c p) i -> p c i", p=128
                    ),
                )
            whps = psW.tile([128, GRP, 66], F32, tag="wh")
            for q in range(GRP):
                mc = mc0 + q
                for kc in range(KCH):
                    nc.tensor.matmul(
                        whps[:, q, :],
                        lhsT=hTs[:, kc, mc * 128 : (mc + 1) * 128],
                        rhs=Waug_bf[:, kc, :],
                        start=(kc == 0),
                        stop=(kc == KCH - 1),
                    )
            sl = slice(mc0, mc0 + GRP)
            nc.scalar.activation(
                gmat[:, sl], whps[:, :, 65], Act.Exp, bias=0.0, scale=0.8
            )
            nc.scalar.activation(
                rhs_aug[:, sl, F_OUT], whps[:, :, 65], Act.Exp, bias=0.0, scale=0.2
            )
            nc.scalar.activation(
                gpmat[:, sl], whps[:, :, 65], Act.Exp, bias=0.0, scale=0.2
            )
            for q in range(GRP):
                mc = mc0 + q
                if RHSDVE:
                    nc.vector.tensor_scalar(
                        rhs_aug[:, mc, 0:F_OUT],
                        whps[:, q, 0:F_OUT],
                        gpmat[:, mc : mc + 1],
                        None,
                        Alu.mult,
                    )
                else:
                    nc.scalar.activation(
                        rhs_aug[:, mc, 0:F_OUT],
                        whps[:, q, 0:F_OUT],
                        Act.Copy,
                        bias=0.0,
                        scale=gpmat[:, mc : mc + 1],
                    )

        def attend(gi):
            # first/last group use per-chunk masks to shorten the pipeline
            # ramp (first acc matmul sooner) and drain (last acc sooner).
            fine = gi in (0, 1, NG - 1)
            mc0 = gi * GRP
            adjt = adjt_tiles.pop(gi)
            Xq = wk.tile([128, GRP, ROWS], BF16, tag="X")
            p4 = wk.tile([128, GRP, ROWS], BF16, tag="p")
            for q in range(GRP):
                mc = mc0 + q
                nc.vector.tensor_scalar(
                    Xq[:, q, :], u_b, gmat[:, mc : mc + 1], 1.0, Alu.mult, Alu.max
                )
                if fine:
                    nc.vector.tensor_tensor(
                        p4[:, q, :], Xq[:, q, :], adjt[:, q, :], Alu.mult
                    )
                    nc.tensor.matmul(
                        accL,
                        lhsT=rhs_aug[:, mc, :],
                        rhs=p4[:, q, 0:512],
                        start=(mc == 0),
                        stop=(mc == MCH - 1),
                    )
                    nc.tensor.matmul(
                        accR,
                        lhsT=rhs_aug[:, mc, :],
                        rhs=p4[:, q, 512:1024],
                        start=(mc == 0),
                        stop=(mc == MCH - 1),
                    )
            if fine:
                return
            if gi % 8 in pool_slots:
                nc.gpsimd.tensor_tensor(p4, Xq, adjt, Alu.mult)
            else:
                nc.vector.tensor_tensor(p4, Xq, adjt, Alu.mult)
            for q in range(GRP):
                mc = mc0 + q
                nc.tensor.matmul(
                    accL,
                    lhsT=rhs_aug[:, mc, :],
                    rhs=p4[:, q, 0:512],
                    start=(mc == 0),
                    stop=(mc == MCH - 1),
                )
                nc.tensor.matmul(
                    accR,
                    lhsT=rhs_aug[:, mc, :],
                    rhs=p4[:, q, 512:1024],
                    start=(mc == 0),
                    stop=(mc == MCH - 1),
                )

        for gi in range(NG + LOOK):
            if gi < NG:
                produce(gi)
            if gi >= LOOK:
                attend(gi - LOOK)

        # ---------------- epilogue: transpose back + softmax-div + ELU ----
        for qd in range(2):
            acc = accL if qd == 0 else accR
            sT4 = ep.tile([65, 512], F32, tag="sT")
            if qd == 0:
                nc.scalar.activation(sT4, acc, Act.Copy)
            else:
                nc.vector.tensor_copy(sT4, acc)
            scr6 = psS.tile([128, 4, 65], F32, tag="s")
            for j in range(4):
                nc.tensor.transpose(
                    scr6[:, j, :], sT4[:, j * 128 : (j + 1) * 128], idf[0:65, 0:65]
                )
            rz4 = ep.tile([128, 4], F32, tag="rz")
            nc.vector.reciprocal(rz4, scr6[:, :, F_OUT])
            sc4 = ep.tile([128, 4, F_OUT], F32, tag="sc")
            rz4bc = rz4.unsqueeze(-1).broadcast_to([128, 4, F_OUT])
            nc.vector.tensor_tensor(sc4, scr6[:, :, 0:F_OUT], rz4bc, Alu.mult)
            mn4 = ep.tile([128, 4, F_OUT], F32, tag="mn")
            nc.vector.tensor_scalar(mn4, sc4, 0.0, None, Alu.min)
            em4 = ep.tile([128, 4, F_OUT], F32, tag="em")
            nc.scalar.activation(em4, mn4, Act.Exp, bias=0.0, scale=1.0)
            # elu(x) = max(exp(min(x,0)) - 1, x)
            ob4 = ep.tile([128, 4, F_OUT], F32, tag="ob")
            nc.vector.scalar_tensor_tensor(ob4, em4, -1.0, sc4, Alu.add, Alu.max)
            oq = nc.scalar if qd == 0 else nc.sync
            oq.dma_start(
                out=out[qd * 512 : (qd + 1) * 512, :].rearrange(
                    "(c p) f -> p c f", p=128
                ),
                in_=ob4,
            )


def _get_nc():
    key = (
        "nc4",
        os.environ.get("GAT_ADJBUFS", ""),
        os.environ.get("GAT_WKBUFS", ""),
        os.environ.get("GAT_SLOTS", ""),
        os.environ.get("GAT_POOL16", ""),
        os.environ.get("GAT_SUBSZ", ""),
        os.environ.get("GAT_RHSDVE", ""),
        os.environ.get("GAT_PBC", ""),
        os.environ.get("GAT_LOOK", ""),
        os.environ.get("GAT_GRP", ""),
    )
    if key not in _CACHE:
        _CACHE[key] = _build_nc()
    return _CACHE[key]


def make_in_maps(h, adj, W, a):
    h = np.ascontiguousarray(h, dtype=np.float32)
    W = np.ascontiguousarray(W, dtype=np.float32)
    a = np.ascontiguousarray(a, dtype=np.float32)

    hT = np.ascontiguousarray(h.T.astype(NPBF16))
    WT = np.ascontiguousarray(W.T)
    in_maps = []
    for c in range(N_CORES):
        sl = slice(c * ROWS, (c + 1) * ROWS)
        in_maps.append(
            {
                "hT": hT,
                "hsT": np.ascontiguousarray(h[sl].T.astype(NPBF16)),
                "adjT": adj[sl].T.astype(NPBF16),
                "W": W,
                "WT": WT,
                "a": a,
            }
        )
    return in_maps


def kernel(h, adj, W, a, _collect_results=False, _trace=False):
    nc = _get_nc()
    in_maps = make_in_maps(h, adj, W, a)
    res = run_bass_kernel_spmd(nc, in_maps, list(range(N_CORES)), trace=_trace)
    out = np.concatenate([res.results[c]["out"] for c in range(N_CORES)], axis=0)
    out = np.ascontiguousarray(out, dtype=np.float32)
    if _collect_results:
        return out, res
    return out
